# revision 13
# baseline (speedup 1.0000x reference)
"""AttMaxPool2D (2x2 softmax-attention pooling) Trainium2 Bass kernel.

Problem: x [16, 224, 224, 128] f32 NHWC -> out [16, 112, 112, 128]
  patches = 2x2 non-overlapping windows; out = sum(p * softmax(p, axis=window)).

Sharding: pure data parallel over batch: 8 cores x 2 examples each.

Per-core layout: partition dim = flattened output row (b_loc*112+ho), free dim
= segments of the input row-pair.  Each chunk loads the even row segment and
the odd row segment (fully contiguous per partition -> 2-dim DMA APs with
4KB-contiguous descriptors), computes exp on ACT, then the softmax-weighted
window sum on DVE:
  out = (A*eA + B*eB + C*eC + D*eD) / (eA+eB+eC+eD)
where A,B = (even row, even/odd col), C,D = (odd row, even/odd col).
"""

import os
from contextlib import ExitStack

import numpy as np

import concourse.bass as bass
import concourse.mybir as mybir
import concourse.tile as tile

F32 = mybir.dt.float32

# Full problem shape (hardcoded per contract).
B, H, W, C = 16, 224, 224, 128
N_CORES = 8
B_LOC = B // N_CORES


def _legalize_waits(nc, max_waits=1):
    """This walrus build's ISA structs accept a single sync-wait command per
    instruction, but Tile's wait emission (not transitively minimal) can leave
    2+ waits.  Two-step fix, semantics-preserving:
      1. prune a wait when it is provably dominated through a kept wait
         (some instruction on the kept wait's engine proc, at/before the kept
         wait value, itself directly waits on the dropped semaphore at >= the
         dropped value);
      2. hoist any remaining extras onto same-engine NoOp instructions
         inserted immediately before (sequencer program order preserves the
         blocking semantics)."""
    import bass_rust
    from concourse.tile_scheduler import PROC_NAME_TO_IDX

    f = nc.m.functions[0]
    insts = [i for b in f.blocks for i in b.instructions]

    def pidx(ant_name):
        return PROC_NAME_TO_IDX[ant_name.rsplit("_", 1)[0]]

    by_proc = {}
    for i in insts:
        p = getattr(i, "bass_scheduled_proc", None)
        t = getattr(i, "bass_scheduled_tick", None)
        if p is None or t is None:
            continue
        by_proc.setdefault(p, []).append((t, i))
    for v in by_proc.values():
        v.sort(key=lambda x: x[0])

    def direct_waits(j):
        si = j.sync_info
        out = {}
        for w in si.on_wait if si else []:
            k = pidx(w.ant_name)
            out[k] = max(out.get(k, -1), w.wait_value)
        return out

    engine_procs = {v for k, v in PROC_NAME_TO_IDX.items()
                    if not k.startswith(("DMAHW", "DMASW", "Collectives"))}

    nop_ctr = [0]
    for b in f.blocks:
        new_insts = []
        for i in b.instructions:
            si = i.sync_info
            if not si or len(si.on_wait) <= max_waits:
                new_insts.append(i)
                continue
            # dedupe per-sem (keep max value)
            best = {}
            for w in si.on_wait:
                k = (w.sync_type, w.id)
                if k not in best or w.wait_value > best[k].wait_value:
                    best[k] = w
            kept = list(best.values())
            # step 1: transitive pruning
            for wd in list(kept):
                if len(kept) <= max_waits:
                    break
                wd_p, wd_v = pidx(wd.ant_name), wd.wait_value
                ok = False
                for via in kept:
                    if via is wd:
                        continue
                    via_p, via_v = pidx(via.ant_name), via.wait_value
                    if via_p not in engine_procs:
                        continue
                    for t, j in by_proc.get(via_p, []):
                        if t > via_v:
                            break
                        if direct_waits(j).get(wd_p, -1) >= wd_v:
                            ok = True
                            break
                    if ok:
                        break
                if ok:
                    kept.remove(wd)
            # step 2: hoist extras onto preceding same-engine NoOps
            while len(kept) > max_waits:
                w = kept.pop(0)
                nop = mybir.InstNoOp(name=f"I-waitnop-{nop_ctr[0]}", ins=[], outs=[])
                nop_ctr[0] += 1
                nop.engine = i.engine
                nop.sync_info = bass_rust.SyncInfo(on_wait=[w], on_update=[])
                new_insts.append(nop)
            si.on_wait = kept
            new_insts.append(i)
        b.instructions = new_insts
    return nc


def build_kernel(b_loc=B_LOC, h=H, w=W, c=C, f=2048, legalize=True):
    """Emit the per-core kernel. f = input-row segment length (elems) per chunk."""
    ho, wo = h // 2, w // 2
    rowlen = w * c          # elems per input row
    outrow = wo * c         # elems per output row
    rp = b_loc * ho         # total output rows in this shard
    assert rowlen % f == 0
    n_seg = rowlen // f
    g = f // 2              # output elems per partition per chunk
    q = f // (2 * c)        # pixel-pairs per segment

    nc = bass.Bass()
    x = nc.declare_dram_parameter("x", [b_loc, h, w, c], F32, isOutput=False)
    y = nc.declare_dram_parameter("y", [b_loc, ho, wo, c], F32, isOutput=True)

    # [rp, parity(2), rowlen]: row-pairs across the whole shard (batch rows
    # are contiguous so (b h) flattens seamlessly).
    xv = x[:].rearrange("b h w c -> (b h) (w c)").rearrange(
        "(hp par) f -> hp par f", par=2
    )
    yv = y[:].rearrange("b h w c -> (b h) (w c)")  # [rp, outrow]

    # partition blocks over output rows
    blocks = []
    p0 = 0
    while p0 < rp:
        pn = min(128, rp - p0)
        blocks.append((p0, pn))
        p0 += pn

    with ExitStack() as ctx:
        tc = ctx.enter_context(tile.TileContext(nc))
        iop = ctx.enter_context(tc.tile_pool(name="io", bufs=3))
        epp = ctx.enter_context(tc.tile_pool(name="ex", bufs=3))
        tmp = ctx.enter_context(tc.tile_pool(name="tmp", bufs=2))

        def quad(t, pn):
            # t: [pn, 2f] -> A,B,Cc,Dd views [pn, q, c]
            v = t[:].rearrange(
                "p (half q two c) -> p half q two c", half=2, q=q, two=2, c=c
            )
            return v[:, 0, :, 0, :], v[:, 0, :, 1, :], v[:, 1, :, 0, :], v[:, 1, :, 1, :]

        mul = mybir.AluOpType.mult
        add = mybir.AluOpType.add

        for p0, pn in blocks:
            for k in range(n_seg):
                xin = iop.tile([pn, 2 * f], F32, tag="xin")
                xin3 = xin[:].rearrange("p (par f) -> p par f", par=2)
                # issue input DMA from the ACT sequencer: the exp's WAR/RAW
                # edges become same-engine (no extra sem waits on the DMA)
                nc.scalar.dma_start(xin3, xv[p0:p0 + pn, :, k * f:(k + 1) * f])

                ex = epp.tile([pn, 2 * f], F32, tag="ex")
                nc.scalar.activation(ex[:], xin[:], mybir.ActivationFunctionType.Exp)

                A, Bv, Cv, Dv = quad(xin, pn)
                EA, EB, EC, ED = quad(ex, pn)

                def t3(tag):
                    t = tmp.tile([pn, g], F32, tag=tag)
                    return t, t[:].rearrange("p (q c) -> p q c", q=q, c=c)

                s1, s1v = t3("s1")
                nc.vector.tensor_tensor(s1v, EA, EB, add)
                s2, s2v = t3("s2")
                nc.vector.tensor_tensor(s2v, EC, ED, add)
                nc.vector.tensor_tensor(s1v, s1v, s2v, add)

                n1, n1v = t3("n1")
                nc.vector.tensor_tensor(n1v, A, EA, mul)
                n2, n2v = t3("n2")
                nc.vector.tensor_tensor(n2v, Bv, EB, mul)
                nc.vector.tensor_tensor(n1v, n1v, n2v, add)
                n3, n3v = t3("n3")
                nc.vector.tensor_tensor(n3v, Cv, EC, mul)
                n4, n4v = t3("n4")
                nc.vector.tensor_tensor(n4v, Dv, ED, mul)
                nc.vector.tensor_tensor(n3v, n3v, n4v, add)
                nc.vector.tensor_tensor(n1v, n1v, n3v, add)

                r, rv = t3("r")
                nc.vector.reciprocal(r[:], s1[:])

                outt, outtv = t3("outt")
                nc.vector.tensor_tensor(outtv, n1v, rv, mul)

                nc.sync.dma_start(yv[p0:p0 + pn, k * g:(k + 1) * g], outt[:])

    return _legalize_waits(nc) if legalize else nc


def kernel(**inputs) -> np.ndarray:
    from concourse.bass_utils import run_bass_kernel_spmd

    x = inputs["x"]
    assert x.shape == (B, H, W, C) and x.dtype == np.float32
    nc = build_kernel()
    shards = x.reshape(N_CORES, B_LOC, H, W, C)
    in_maps = [{"x": np.ascontiguousarray(shards[i])} for i in range(N_CORES)]
    res = run_bass_kernel_spmd(nc, in_maps, list(range(N_CORES)))
    return np.concatenate([r["y"] for r in res.results], axis=0)


if __name__ == "__main__":
    # Small-shape CoreSim validation (no hardware).
    from concourse.bass_interp import CoreSim

    b_loc, h, w, c, f = 1, 8, 16, 128, 1024
    nc = build_kernel(b_loc, h, w, c, f, legalize=False)
    rng = np.random.default_rng(0)
    xs = rng.standard_normal((b_loc, h, w, c), dtype=np.float32)

    sim = CoreSim(nc)
    sim.tensor("x")[:] = xs
    sim.simulate()
    got = sim.tensor("y").copy()

    xd = xs.astype(np.float64)
    p = xd.reshape(b_loc, h // 2, 2, w // 2, 2, c).transpose(0, 1, 3, 2, 4, 5)
    p = p.reshape(b_loc, h // 2, w // 2, 4, c)
    e = np.exp(p - p.max(axis=3, keepdims=True))
    ref = (p * e).sum(axis=3) / e.sum(axis=3)
    err = np.abs(got - ref).max() / np.abs(ref).max()
    print("scale-rel err:", err, "max abs err:", np.abs(got - ref).max())
    assert err < 1e-5, "sim mismatch"
    print("SIM OK")


# revision 14
# speedup vs baseline: 1.4847x; 1.4847x over previous
"""AttMaxPool2D (2x2 softmax-attention pooling) Trainium2 Bass kernel.

Problem: x [16, 224, 224, 128] f32 NHWC -> out [16, 112, 112, 128]
  patches = 2x2 non-overlapping windows; out = sum(p * softmax(p, axis=window)).

Sharding: pure data parallel over batch: 8 cores x 2 examples each.

Per-core layout: partition dim = flattened output row (b_loc*112+ho), free dim
= segments of the input row-pair.  Each chunk loads the even row segment and
the odd row segment (fully contiguous per partition -> 2-dim DMA APs with
4KB-contiguous descriptors), computes exp on ACT, then the softmax-weighted
window sum on DVE:
  out = (A*eA + B*eB + C*eC + D*eD) / (eA+eB+eC+eD)
where A,B = (even row, even/odd col), C,D = (odd row, even/odd col).
"""

import os
from contextlib import ExitStack

import numpy as np

import concourse.bass as bass
import concourse.mybir as mybir
import concourse.tile as tile

F32 = mybir.dt.float32

# Full problem shape (hardcoded per contract).
B, H, W, C = 16, 224, 224, 128
N_CORES = 8
B_LOC = B // N_CORES


def _legalize_waits(nc, max_waits=1):
    """This walrus build's ISA structs accept a single sync-wait command per
    instruction, but Tile's wait emission (not transitively minimal) can leave
    2+ waits.  Two-step fix, semantics-preserving:
      1. prune a wait when it is provably dominated through a kept wait
         (some instruction on the kept wait's engine proc, at/before the kept
         wait value, itself directly waits on the dropped semaphore at >= the
         dropped value);
      2. hoist any remaining extras onto same-engine NoOp instructions
         inserted immediately before (sequencer program order preserves the
         blocking semantics)."""
    import bass_rust
    from concourse.tile_scheduler import PROC_NAME_TO_IDX

    f = nc.m.functions[0]
    insts = [i for b in f.blocks for i in b.instructions]

    def pidx(ant_name):
        return PROC_NAME_TO_IDX[ant_name.rsplit("_", 1)[0]]

    by_proc = {}
    for i in insts:
        p = getattr(i, "bass_scheduled_proc", None)
        t = getattr(i, "bass_scheduled_tick", None)
        if p is None or t is None:
            continue
        by_proc.setdefault(p, []).append((t, i))
    for v in by_proc.values():
        v.sort(key=lambda x: x[0])

    def direct_waits(j):
        si = j.sync_info
        out = {}
        for w in si.on_wait if si else []:
            k = pidx(w.ant_name)
            out[k] = max(out.get(k, -1), w.wait_value)
        return out

    engine_procs = {v for k, v in PROC_NAME_TO_IDX.items()
                    if not k.startswith(("DMAHW", "DMASW", "Collectives"))}

    nop_ctr = [0]
    for b in f.blocks:
        new_insts = []
        for i in b.instructions:
            si = i.sync_info
            if not si or len(si.on_wait) <= max_waits:
                new_insts.append(i)
                continue
            # dedupe per-sem (keep max value)
            best = {}
            for w in si.on_wait:
                k = (w.sync_type, w.id)
                if k not in best or w.wait_value > best[k].wait_value:
                    best[k] = w
            kept = list(best.values())
            # step 1: transitive pruning
            for wd in list(kept):
                if len(kept) <= max_waits:
                    break
                wd_p, wd_v = pidx(wd.ant_name), wd.wait_value
                ok = False
                for via in kept:
                    if via is wd:
                        continue
                    via_p, via_v = pidx(via.ant_name), via.wait_value
                    if via_p not in engine_procs:
                        continue
                    for t, j in by_proc.get(via_p, []):
                        if t > via_v:
                            break
                        if direct_waits(j).get(wd_p, -1) >= wd_v:
                            ok = True
                            break
                    if ok:
                        break
                if ok:
                    kept.remove(wd)
            # step 2: hoist extras onto preceding same-engine NoOps
            while len(kept) > max_waits:
                w = kept.pop(0)
                nop = mybir.InstNoOp(name=f"I-waitnop-{nop_ctr[0]}", ins=[], outs=[])
                nop_ctr[0] += 1
                nop.engine = i.engine
                nop.sync_info = bass_rust.SyncInfo(on_wait=[w], on_update=[])
                new_insts.append(nop)
            si.on_wait = kept
            new_insts.append(i)
        b.instructions = new_insts
    return nc


def build_kernel(b_loc=B_LOC, h=H, w=W, c=C, f=2048, legalize=True):
    """Emit the per-core kernel. f = input-row segment length (elems) per chunk."""
    ho, wo = h // 2, w // 2
    rowlen = w * c          # elems per input row
    outrow = wo * c         # elems per output row
    rp = b_loc * ho         # total output rows in this shard
    assert rowlen % f == 0
    n_seg = rowlen // f
    g = f // 2              # output elems per partition per chunk
    q = f // (2 * c)        # pixel-pairs per segment

    nc = bass.Bass()
    x = nc.declare_dram_parameter("x", [b_loc, h, w, c], F32, isOutput=False)
    y = nc.declare_dram_parameter("y", [b_loc, ho, wo, c], F32, isOutput=True)

    # [rp, parity(2), rowlen]: row-pairs across the whole shard (batch rows
    # are contiguous so (b h) flattens seamlessly).
    xv = x[:].rearrange("b h w c -> (b h) (w c)").rearrange(
        "(hp par) f -> hp par f", par=2
    )
    yv = y[:].rearrange("b h w c -> (b h) (w c)")  # [rp, outrow]

    # partition blocks over output rows
    blocks = []
    p0 = 0
    while p0 < rp:
        pn = min(128, rp - p0)
        blocks.append((p0, pn))
        p0 += pn

    with ExitStack() as ctx:
        tc = ctx.enter_context(tile.TileContext(nc))
        iop = ctx.enter_context(tc.tile_pool(name="io", bufs=3))
        epp = ctx.enter_context(tc.tile_pool(name="ex", bufs=3))
        tmp = ctx.enter_context(tc.tile_pool(name="tmp", bufs=2))

        def quad(t, pn):
            # t: [pn, 2f] -> A,B,Cc,Dd views [pn, q, c]
            v = t[:].rearrange(
                "p (half q two c) -> p half q two c", half=2, q=q, two=2, c=c
            )
            return v[:, 0, :, 0, :], v[:, 0, :, 1, :], v[:, 1, :, 0, :], v[:, 1, :, 1, :]

        mul = mybir.AluOpType.mult
        add = mybir.AluOpType.add

        for p0, pn in blocks:
            for k in range(n_seg):
                xin = iop.tile([pn, 2 * f], F32, tag="xin")
                xin3 = xin[:].rearrange("p (par f) -> p par f", par=2)
                # issue input DMA from the ACT sequencer: the exp's WAR/RAW
                # edges become same-engine (no extra sem waits on the DMA)
                nc.scalar.dma_start(xin3, xv[p0:p0 + pn, :, k * f:(k + 1) * f])

                ex = epp.tile([pn, 2 * f], F32, tag="ex")
                nc.scalar.activation(ex[:], xin[:], mybir.ActivationFunctionType.Exp)

                A, Bv, Cv, Dv = quad(xin, pn)
                EA, EB, EC, ED = quad(ex, pn)

                def t3(tag):
                    t = tmp.tile([pn, g], F32, tag=tag)
                    return t, t[:].rearrange("p (q c) -> p q c", q=q, c=c)

                s1, s1v = t3("s1")
                nc.vector.tensor_tensor(s1v, EA, EB, add)
                s2, s2v = t3("s2")
                nc.vector.tensor_tensor(s2v, EC, ED, add)
                nc.vector.tensor_tensor(s1v, s1v, s2v, add)

                n1, n1v = t3("n1")
                nc.vector.tensor_tensor(n1v, A, EA, mul)
                n2, n2v = t3("n2")
                nc.vector.tensor_tensor(n2v, Bv, EB, mul)
                nc.vector.tensor_tensor(n1v, n1v, n2v, add)
                n3, n3v = t3("n3")
                nc.vector.tensor_tensor(n3v, Cv, EC, mul)
                n4, n4v = t3("n4")
                nc.vector.tensor_tensor(n4v, Dv, ED, mul)
                nc.vector.tensor_tensor(n3v, n3v, n4v, add)
                nc.vector.tensor_tensor(n1v, n1v, n3v, add)

                # 1/s on ACT: r = exp(-ln(s)) — Ln and Exp share one table
                # set (natural_log_exp_and_others); keeps DVE free of the
                # ~6 cyc/elem iterative divide.
                lns, _ = t3("lns")
                nc.scalar.activation(lns[:], s1[:], mybir.ActivationFunctionType.Ln)
                r, rv = t3("r")
                nc.scalar.activation(r[:], lns[:], mybir.ActivationFunctionType.Exp,
                                     scale=-1.0)

                outt, outtv = t3("outt")
                nc.vector.tensor_tensor(outtv, n1v, rv, mul)

                nc.sync.dma_start(yv[p0:p0 + pn, k * g:(k + 1) * g], outt[:])

    return _legalize_waits(nc) if legalize else nc


def kernel(**inputs) -> np.ndarray:
    from concourse.bass_utils import run_bass_kernel_spmd

    x = inputs["x"]
    assert x.shape == (B, H, W, C) and x.dtype == np.float32
    nc = build_kernel()
    shards = x.reshape(N_CORES, B_LOC, H, W, C)
    in_maps = [{"x": np.ascontiguousarray(shards[i])} for i in range(N_CORES)]
    res = run_bass_kernel_spmd(nc, in_maps, list(range(N_CORES)))
    return np.concatenate([r["y"] for r in res.results], axis=0)


if __name__ == "__main__":
    # Small-shape CoreSim validation (no hardware).
    from concourse.bass_interp import CoreSim

    b_loc, h, w, c, f = 1, 8, 16, 128, 1024
    nc = build_kernel(b_loc, h, w, c, f, legalize=False)
    rng = np.random.default_rng(0)
    xs = rng.standard_normal((b_loc, h, w, c), dtype=np.float32)

    sim = CoreSim(nc)
    sim.tensor("x")[:] = xs
    sim.simulate()
    got = sim.tensor("y").copy()

    xd = xs.astype(np.float64)
    p = xd.reshape(b_loc, h // 2, 2, w // 2, 2, c).transpose(0, 1, 3, 2, 4, 5)
    p = p.reshape(b_loc, h // 2, w // 2, 4, c)
    e = np.exp(p - p.max(axis=3, keepdims=True))
    ref = (p * e).sum(axis=3) / e.sum(axis=3)
    err = np.abs(got - ref).max() / np.abs(ref).max()
    print("scale-rel err:", err, "max abs err:", np.abs(got - ref).max())
    assert err < 1e-5, "sim mismatch"
    print("SIM OK")
